# revision 1
# baseline (speedup 1.0000x reference)
"""Multi-head attention (B=8, N=1024, C=768, H=12) on 8 Trainium2 NeuronCores.

Sharding: data-parallel over batch — one batch element per core, no collectives.

Per-core dataflow (all layouts chosen so NO on-chip transposes are needed):
  - Host pre-transposes x and the weights into contraction-on-partition layouts.
  - Q^T,K^T computed in [o, n] layout (o on partitions), V in [n, o] layout with a
    65-stride per-head interleave whose 65th column is set to 1.0 (ones-augmented
    V) so the attn@V matmul also produces the softmax row-sums for free.
  - S^T[m, n] = K^T.T @ Q^T per head (contraction over d=64 on partitions).
  - P^T = exp(0.125 * S^T) on ScalarE (no max-subtraction: logits ~ N(0,1)).
  - O'[d, n] (+rowsum row) = V'aug.T @ P^T, accumulated over m-tiles in PSUM.
  - normalize: broadcast 1/rowsum over partitions (SWDGE stride-0 DMA) and
    multiply; store O' in [c, n] layout (two heads stacked per 128-partition tile).
  - final = O'.T @ proj_w^T + bias in [n, c'] layout, DMA'd out.
All matmuls run as float32r (full-rate single-pass) with fp32 PSUM accumulation.
"""

import numpy as np

_STATE = {}

B, N, C = 8, 1024, 768
H, D = 12, 64
KT = 6           # contraction tiles of 128 over C
P = 128
NT = N // P      # 8 n-tiles
PAIRS = H // 2   # 6 head pairs
VW = H * (D + 1)  # 780: ones-augmented per-head V width


def _patch_tile_drain():
    """This walrus build rejects >1 sem wait on a CTRL (Drain) instruction.

    TileContext's exit puts one wait per outstanding semaphore on the final SP
    Drain; redistribute them across single-wait NOPs preceding the drain.
    """
    import bass_rust
    import concourse.tile as tile
    from concourse.vector_clock import ScopedClock

    if getattr(tile.TileContext, "_ant_drain_patched", False):
        return

    SyncInfo = bass_rust.SyncInfo

    def _drain_and_barrier(self, tick_clock, wait_clock):
        nc = self.nc
        probe = nc.sync.nop(nofuse=True)
        wait_clock.add_sem_waits(
            probe.ins, ScopedClock({None: tick_clock.global_clock})
        )
        si = probe.ins.sync_info
        waits = list(si.on_wait or []) if si is not None else []
        updates = list(si.on_update or []) if si is not None else []
        if len(waits) > 1:
            probe.ins.sync_info = SyncInfo(on_wait=waits[:1], on_update=updates)
            for w in waits[1:]:
                extra = nc.sync.nop(nofuse=True)
                extra.ins.sync_info = SyncInfo(on_wait=[w], on_update=[])
        nc.sync.drain()

        nc.all_engine_barrier()
        assert self.sems is not None
        popped = nc._tile_sem_poison_stack.pop()
        assert popped is self._sem_poison
        nc.clear_and_free_semaphores(list(self.sems.allocated().values()))
        nc.all_engine_barrier()

    tile.TileContext._drain_and_barrier = _drain_and_barrier
    tile.TileContext._ant_drain_patched = True


def _split_multi_waits(nc):
    """This walrus build allows at most ONE sem wait per instruction.

    Tile's wait assignment routinely puts several; hoist all but the last onto
    single-wait NOPs inserted immediately before the instruction on the same
    engine (engines execute block instructions in order, so semantics are
    unchanged).
    """
    from concourse import mybir

    for fn in nc.m.functions:
        for bb in fn.blocks:
            out, changed = [], False
            for inst in bb.instructions:
                si = inst.sync_info
                waits = list(si.on_wait) if (si is not None and si.on_wait) else []
                if len(waits) > 1:
                    changed = True
                    for w in waits[:-1]:
                        nop = mybir.InstNoOp(
                            name=f"I-ws{nc.next_id()}",
                            engine=inst.engine,
                            bass_nofuse=True,
                            sync_info=mybir.SyncInfo(on_wait=[w], on_update=[]),
                        )
                        nc.register_instruction(nop)
                        out.append(nop)
                    inst.sync_info = mybir.SyncInfo(
                        on_wait=[waits[-1]], on_update=list(si.on_update or [])
                    )
                out.append(inst)
            if changed:
                bb.instructions = out


def _build_nc(trace_sim=False):
    from contextlib import ExitStack

    import concourse.bass as bass
    import concourse.tile as tile
    from concourse import mybir

    _patch_tile_drain()

    f32 = mybir.dt.float32
    f32r = mybir.dt.float32r

    nc = bass.Bass("TRN2", target_bir_lowering=False, debug=False, num_devices=1)

    xT = nc.dram_tensor("xT", [KT, P, N], f32r, kind="ExternalInput").ap()
    wqk = nc.dram_tensor("wqk", [PAIRS, P, KT * 256], f32r, kind="ExternalInput").ap()
    wv = nc.dram_tensor("wv", [P, KT, VW], f32r, kind="ExternalInput").ap()
    pT = nc.dram_tensor("pT", [P, KT, C], f32r, kind="ExternalInput").ap()
    bias = nc.dram_tensor("bias", [P, C], f32, kind="ExternalInput").ap()
    ones = nc.dram_tensor("ones", [P, H], f32r, kind="ExternalInput").ap()
    pt5hi = nc.dram_tensor("pt5hi", [D, C], f32r, kind="ExternalInput").ap()
    y = nc.dram_tensor("y", [N, C], f32, kind="ExternalOutput").ap()

    Exp = mybir.ActivationFunctionType.Exp
    SCALE = float(D) ** -0.5

    with tile.TileContext(nc, trace_sim=trace_sim) as tc, ExitStack() as ctx:
        kilo = ctx.enter_context(tc.tile_pool(name="kilo", bufs=6))      # xT
        op_ = ctx.enter_context(tc.tile_pool(name="op", bufs=6))        # O'
        qkp = ctx.enter_context(tc.tile_pool(name="qk", bufs=4))
        bigp = ctx.enter_context(tc.tile_pool(name="big", bufs=2))       # wv, pT
        wqkp = ctx.enter_context(tc.tile_pool(name="wqk", bufs=2))
        vp = ctx.enter_context(tc.tile_pool(name="v", bufs=8))
        ptp = ctx.enter_context(tc.tile_pool(name="pt", bufs=4))
        rbp = ctx.enter_context(tc.tile_pool(name="rb", bufs=3))
        tbp = ctx.enter_context(tc.tile_pool(name="tb", bufs=1))
        outp = ctx.enter_context(tc.tile_pool(name="out", bufs=8))
        onep = ctx.enter_context(tc.tile_pool(name="one", bufs=1))
        drp = ctx.enter_context(tc.tile_pool(name="dr", bufs=2, space="DRAM"))
        ps = ctx.enter_context(tc.tile_pool(name="ps", bufs=4, space="PSUM"))

        # warm the ACT exp table set while input DMAs run (the first real exp
        # otherwise pays the ~2.7us ACT_TABLE_LOAD on the critical path)
        warm = onep.tile([1, 4], f32)
        nc.vector.memset(warm[:], 0.0)
        warm2 = onep.tile([1, 4], f32)
        nc.scalar.activation(warm2[:], warm[:], Exp)

        # ---- load constants / inputs ----
        xs = []
        for k in range(KT):
            t = kilo.tile([P, N], f32r, tag="kilo")
            eng = nc.sync if k % 2 == 0 else nc.gpsimd
            eng.dma_start(t[:, 0:512], xT[k][:, 0:512])
            xs.append(t)
        for k in range(KT):
            eng = nc.sync if k % 2 == 0 else nc.gpsimd
            eng.dma_start(xs[k][:, 512:1024], xT[k][:, 512:1024])

        qt_sb, kt_sb = [], []
        wq_tiles = {}

        def prefetch_wq(t):
            if t not in wq_tiles:
                wq_t = wqkp.tile([P, KT * 256], f32r, tag="wqk", name=f"wq_{t}")
                nc.sync.dma_start(wq_t[:], wqk[t])
                wq_tiles[t] = wq_t

        def emit_qk_one(t, which, store):
            prefetch_wq(t)
            wq_t = wq_tiles[t]
            slot = ps.tile([P, 1024], f32, tag="ps")
            for ns in range(2):
                dst = slot[:, ns * 512 : (ns + 1) * 512]
                for k in range(KT):
                    nc.tensor.matmul(
                        dst,
                        wq_t[:, k * 256 + which * P : k * 256 + (which + 1) * P],
                        xs[k][:, ns * 512 : (ns + 1) * 512],
                        start=(k == 0),
                        stop=(k == KT - 1),
                    )
            qk_t = qkp.tile([P, N], f32r, tag="qk")
            nc.vector.tensor_copy(qk_t[:], slot[:, 0:1024])
            store.append(qk_t)

        # QK of pair 0 first so attention can start early
        prefetch_wq(0)
        emit_qk_one(0, 0, qt_sb)
        emit_qk_one(0, 1, kt_sb)

        wv_sb = bigp.tile([P, KT, VW], f32r, tag="big")
        nc.gpsimd.dma_start(wv_sb[:], wv[:])

        bias_sb = onep.tile([P, C], f32)
        nc.gpsimd.dma_start(bias_sb[:], bias[:])
        pt5hi_sb = onep.tile([D, C], f32r)
        nc.gpsimd.dma_start(pt5hi_sb[:], pt5hi[:])
        tb5p = ctx.enter_context(tc.tile_pool(name="tb5", bufs=1))
        tb5_holder = []

        # ---- V' in [n, 780] layout (ones-augmented heads), emitted just-in-time ----
        v_sb = []

        def emit_v(nt):
            slot = ps.tile([P, 1024], f32, tag="ps")
            for half, (c0, w) in enumerate(((0, 390), (390, 390))):
                dst = slot[:, half * 512 : half * 512 + w]
                for k in range(KT):
                    nc.tensor.matmul(
                        dst,
                        xs[k][:, nt * P : (nt + 1) * P],
                        wv_sb[:, k, c0 : c0 + w],
                        start=(k == 0),
                        stop=(k == KT - 1),
                    )
            vt = vp.tile([P, VW], f32r, tag="v")
            nc.vector.tensor_copy(vt[:, 0:390], slot[:, 0:390])
            nc.vector.tensor_copy(vt[:, 390:780], slot[:, 512:902])
            ones_ap = vt.rearrange("p (h w) -> p h w", w=D + 1)[:, :, D]
            nc.sync.dma_start(ones_ap, ones[:])
            v_sb.append(vt)

        emit_v(0)
        emit_qk_one(1, 0, qt_sb)
        emit_qk_one(1, 1, kt_sb)

        # ---- attention, one head at a time; V'/QK of upcoming work interleaved ----
        o_sb = []
        pt_w = None
        for t in range(PAIRS):
            if t == 1:
                pt_w = bigp.tile([P, KT, C], f32r, tag="big")
                nc.gpsimd.dma_start(pt_w[:], pT[:])
            ot = op_.tile([P, N], f32r, tag="op")
            for head in range(2):
                hb = head * D
                h = 2 * t + head
                o_slot = ps.tile([P, 1024], f32, tag="ps")
                for j in range(NT):
                    s_slot = ps.tile([P, 1024], f32, tag="ps")
                    for ns in range(2):
                        nc.tensor.matmul(
                            s_slot[:, ns * 512 : (ns + 1) * 512],
                            kt_sb[t][hb : hb + D, j * P : (j + 1) * P],
                            qt_sb[t][hb : hb + D, ns * 512 : (ns + 1) * 512],
                            start=True,
                            stop=True,
                        )
                    pt_t = ptp.tile([P, 1024], f32r, tag="pt")
                    nc.scalar.activation(pt_t[:], s_slot[:], Exp, scale=SCALE)
                    for ns in range(2):
                        nc.tensor.matmul(
                            o_slot[0 : D + 1, ns * 512 : (ns + 1) * 512],
                            v_sb[j][:, h * (D + 1) : (h + 1) * (D + 1)],
                            pt_t[:, ns * 512 : (ns + 1) * 512],
                            start=(j == 0),
                            stop=(j == NT - 1),
                            skip_group_check=True,
                        )
                    if t == 0 and head == 0 and j + 1 < NT:
                        emit_v(j + 1)
                    if head == 0 and t + 1 < PAIRS and j == 6:
                        prefetch_wq(t + 1)
                    if head == 1 and 1 <= t < PAIRS - 1:
                        if j == 2:
                            emit_qk_one(t + 1, 0, qt_sb)
                        elif j == 5:
                            emit_qk_one(t + 1, 1, kt_sb)
                # normalize: O'[0:64] / rowsum (row 64); heads stacked in ot
                rb = rbp.tile([P, N], f32, tag="rb")
                nc.vector.tensor_copy(rb[D : D + 1, :], o_slot[D : D + 1, :])
                scratch = drp.tile([1, N], f32, tag="dr")
                nc.sync.dma_start(scratch[0:1, :], rb[D : D + 1, :])
                bcast_src = bass.AP(
                    tensor=scratch.tensor,
                    offset=scratch.offset,
                    ap=[[0, D]] + [list(dd) for dd in scratch[0:1, :].ap[1:]],
                )
                nc.gpsimd.dma_start(out=rb[0:D, :], in_=bcast_src)
                nc.vector.reciprocal(rb[0:D, :], rb[0:D, :])
                if head == 0:
                    nc.vector.tensor_mul(ot[0:D, :], o_slot[0:D, :], rb[0:D, :])
                elif t == PAIRS - 1:
                    tb5 = tb5p.tile([D, N], f32r, tag="tb5")
                    nc.vector.tensor_mul(tb5[:], o_slot[0:D, :], rb[0:D, :])
                    tb5_holder.append(tb5)
                else:
                    tb = tbp.tile([D, N], f32r, tag="tb")
                    nc.vector.tensor_mul(tb[:], o_slot[0:D, :], rb[0:D, :])
                    nc.sync.dma_start(ot[D:P, :], tb[:])
            o_sb.append(ot)

        # ---- projection + bias ----
        # k=5 depends on the last pair's normalize; accumulate k=0..4 (+bias)
        # into SBUF per n-tile first (frees the PSUM unit immediately, so all
        # eight partials overlap the last pair), then only the two half-K k=5
        # matmuls and a final add remain on the exposed tail.
        acc_sb = {}

        def proj_partial(nt):
            slot = ps.tile([P, 1024], f32, tag="ps", name=f"proj_{nt}")
            for k in range(KT - 1):
                for c0, w in ((0, 512), (512, 256)):
                    nc.tensor.matmul(
                        slot[:, c0 : c0 + w],
                        o_sb[k][:, nt * P : (nt + 1) * P],
                        pt_w[:, k, c0 : c0 + w],
                        start=(k == 0),
                        stop=(k == KT - 2),
                        skip_group_check=True,
                    )
            acc = outp.tile([P, C], f32, tag="out", name=f"acc_{nt}")
            nc.vector.tensor_add(acc[:], slot[:, 0:C], bias_sb[:])
            acc_sb[nt] = acc

        def proj_finish(nt):
            slot = ps.tile([P, 1024], f32, tag="ps", name=f"projf_{nt}")
            k = KT - 1
            for c0, w in ((0, 512), (512, 256)):
                nc.tensor.matmul(
                    slot[:, c0 : c0 + w],
                    o_sb[k][0:D, nt * P : (nt + 1) * P],
                    pt_w[0:D, k, c0 : c0 + w],
                    start=True,
                    stop=False,
                    skip_group_check=True,
                )
            for c0, w in ((0, 512), (512, 256)):
                nc.tensor.matmul(
                    slot[:, c0 : c0 + w],
                    tb5_holder[0][:, nt * P : (nt + 1) * P],
                    pt5hi_sb[:, c0 : c0 + w],
                    start=False,
                    stop=True,
                    skip_group_check=True,
                )
            acc = acc_sb.pop(nt)
            nc.vector.tensor_add(acc[:], acc[:], slot[:, 0:C])
            nc.sync.dma_start(y[nt * P : (nt + 1) * P, :], acc[:])

        for nt in range(NT):
            proj_partial(nt)
        for nt in range(NT):
            proj_finish(nt)

    _split_multi_waits(nc)
    return nc


def _prep_shared(qkv_w, proj_w, proj_b):
    f = np.float32
    wq = qkv_w[0:C].astype(f)          # [o, c]
    wk = qkv_w[C : 2 * C].astype(f)
    wv_ = qkv_w[2 * C : 3 * C].astype(f)
    wqT, wkT, wvT = wq.T.copy(), wk.T.copy(), wv_.T.copy()  # [c, o]

    wqk = np.zeros((PAIRS, P, KT, 256), f)
    for t in range(PAIRS):
        for k in range(KT):
            wqk[t, :, k, 0:P] = wqT[k * P : (k + 1) * P, t * P : (t + 1) * P]
            wqk[t, :, k, P:256] = wkT[k * P : (k + 1) * P, t * P : (t + 1) * P]
    wqk = wqk.reshape(PAIRS, P, KT * 256)

    wvh = np.zeros((P, KT, H, D + 1), f)
    for k in range(KT):
        wvh[:, k, :, 0:D] = wvT[k * P : (k + 1) * P].reshape(P, H, D)
    wvh = wvh.reshape(P, KT, VW)

    pTh = proj_w.T.astype(f).reshape(KT, P, C).transpose(1, 0, 2).copy()
    pt5hi = np.ascontiguousarray(proj_w.T.astype(f)[C - D : C, :])
    bias_h = np.ascontiguousarray(np.broadcast_to(proj_b.astype(f), (P, C)))
    return wqk, wvh, pTh, bias_h, pt5hi


def kernel(x, qkv_w, proj_w, proj_b):
    from concourse.bass_utils import run_bass_kernel_spmd

    x = np.asarray(x, np.float32)
    wqk, wvh, pTh, bias_h, pt5hi = _prep_shared(
        np.asarray(qkv_w), np.asarray(proj_w), np.asarray(proj_b)
    )

    if "nc" not in _STATE:
        _STATE["nc"] = _build_nc()
    nc = _STATE["nc"]

    in_maps = []
    for b in range(B):
        xTb = np.ascontiguousarray(x[b].T).reshape(KT, P, N)
        in_maps.append(
            {"xT": xTb, "wqk": wqk, "wv": wvh, "pT": pTh, "bias": bias_h,
             "ones": np.ones((P, H), np.float32), "pt5hi": pt5hi}
        )

    res = run_bass_kernel_spmd(nc, in_maps, core_ids=list(range(B)))
    return np.stack([res.results[b]["y"] for b in range(B)], axis=0)



# revision 7
# speedup vs baseline: 1.3496x; 1.3496x over previous
"""Multi-head attention (B=8, N=1024, C=768, H=12) on 8 Trainium2 NeuronCores.

Sharding: data-parallel over batch — one batch element per core, no collectives.

Per-core dataflow (v2 — attn@V reoriented to halve its PE column count):
  - All matmul operands are bf16 (fp32 PSUM accumulation); fp32 only for
    bias/psum/normalization. Halves DMA and avoids the fp32r <256-col penalty.
  - Q^T,K^T in [o, n] layout (o on partitions); V in [m, o] layout (plain
    h-major head columns, no augmentation).
  - S^T[m, n] = K^T.T @ Q^T per head (contraction over d=64 on partitions).
  - P^T = exp(0.125 * S^T) on ScalarE, bf16 out (no max-subtraction:
    logits ~ N(0,1)).
  - attn@V in [n, d] orientation: out[n, d] (+= over m-tiles) with the P^T
    128x128 chunk as the *stationary* operand and V[m-tile, head] as the
    64-wide moving operand — 64 cols/m-tile instead of 1024: ~half the PE
    columns of the [d, n] orientation. Softmax row-sums from extra 1-col
    matmuls against a ones vector (free in the cost model).
  - normalize on DVE: one reciprocal + one broadcast-multiply per head,
    writing o_big[n, pair, nt, c] bf16.
  - per-pair DMA-engine transpose (InstDmaTransposeAnt) of o_big pair slab
    [128n x 1024(nt,c)] -> oT[c, pair, nt, n]: zero PE/DVE cost.
  - proj y[n, c'] = sum_cb oT_cb.T @ pT_cb + bias, split into a k=0..4
    partial (overlapped with the last pair) and a k=5 finish.
"""

import numpy as np

_STATE = {}

B, N, C = 8, 1024, 768
H, D = 12, 64
KT = 6           # contraction tiles of 128 over C
P = 128
NT = N // P      # 8 n-tiles
PAIRS = H // 2   # 6 head pairs


def _patch_tile_drain():
    """This walrus build rejects >1 sem wait on a CTRL (Drain) instruction.

    TileContext's exit puts one wait per outstanding semaphore on the final SP
    Drain; redistribute them across single-wait NOPs preceding the drain.
    """
    import bass_rust
    import concourse.tile as tile
    from concourse.vector_clock import ScopedClock

    if getattr(tile.TileContext, "_ant_drain_patched", False):
        return

    SyncInfo = bass_rust.SyncInfo

    def _drain_and_barrier(self, tick_clock, wait_clock):
        nc = self.nc
        probe = nc.sync.nop(nofuse=True)
        wait_clock.add_sem_waits(
            probe.ins, ScopedClock({None: tick_clock.global_clock})
        )
        si = probe.ins.sync_info
        waits = list(si.on_wait or []) if si is not None else []
        updates = list(si.on_update or []) if si is not None else []
        if len(waits) > 1:
            probe.ins.sync_info = SyncInfo(on_wait=waits[:1], on_update=updates)
            for w in waits[1:]:
                extra = nc.sync.nop(nofuse=True)
                extra.ins.sync_info = SyncInfo(on_wait=[w], on_update=[])
        nc.sync.drain()

        nc.all_engine_barrier()
        assert self.sems is not None
        popped = nc._tile_sem_poison_stack.pop()
        assert popped is self._sem_poison
        nc.clear_and_free_semaphores(list(self.sems.allocated().values()))
        nc.all_engine_barrier()

    tile.TileContext._drain_and_barrier = _drain_and_barrier
    tile.TileContext._ant_drain_patched = True


def _split_multi_waits(nc):
    """This walrus build allows at most ONE sem wait per instruction.

    Tile's wait assignment routinely puts several; hoist all but the last onto
    single-wait NOPs inserted immediately before the instruction on the same
    engine (engines execute block instructions in order, so semantics are
    unchanged).
    """
    from concourse import mybir

    for fn in nc.m.functions:
        for bb in fn.blocks:
            out, changed = [], False
            for inst in bb.instructions:
                si = inst.sync_info
                waits = list(si.on_wait) if (si is not None and si.on_wait) else []
                if len(waits) > 1:
                    changed = True
                    for w in waits[:-1]:
                        nop = mybir.InstNoOp(
                            name=f"I-ws{nc.next_id()}",
                            engine=inst.engine,
                            bass_nofuse=True,
                            sync_info=mybir.SyncInfo(on_wait=[w], on_update=[]),
                        )
                        nc.register_instruction(nop)
                        out.append(nop)
                    inst.sync_info = mybir.SyncInfo(
                        on_wait=[waits[-1]], on_update=list(si.on_update or [])
                    )
                out.append(inst)
            if changed:
                bb.instructions = out


def _build_nc(trace_sim=False, debug=False):
    from contextlib import ExitStack

    import concourse.bass as bass
    import concourse.tile as tile
    from concourse import mybir

    _patch_tile_drain()

    f32 = mybir.dt.float32
    bf16 = mybir.dt.bfloat16

    nc = bass.Bass("TRN2", target_bir_lowering=False, debug=False, num_devices=1)

    xT = nc.dram_tensor("xT", [KT, P, N], bf16, kind="ExternalInput").ap()
    wqk = nc.dram_tensor("wqk", [PAIRS, P, KT * 256], bf16, kind="ExternalInput").ap()
    wv = nc.dram_tensor("wv", [P, KT, C], bf16, kind="ExternalInput").ap()
    pT = nc.dram_tensor("pT", [P, KT, C], bf16, kind="ExternalInput").ap()
    bias = nc.dram_tensor("bias", [P, C], f32, kind="ExternalInput").ap()
    y = nc.dram_tensor("y", [N, C], f32, kind="ExternalOutput").ap()
    if debug:
        dbg = {
            "dq": nc.dram_tensor("dq", [P, N], bf16, kind="ExternalOutput").ap(),
            "dk": nc.dram_tensor("dk", [P, N], bf16, kind="ExternalOutput").ap(),
            "dv": nc.dram_tensor("dv", [P, C], bf16, kind="ExternalOutput").ap(),
            "drs": nc.dram_tensor("drs", [P, H * NT], f32, kind="ExternalOutput").ap(),
            "dob": nc.dram_tensor(
                "dob", [P, PAIRS * NT * P], bf16, kind="ExternalOutput"
            ).ap(),
            "dot": nc.dram_tensor(
                "dot", [P, PAIRS * NT * P], bf16, kind="ExternalOutput"
            ).ap(),
        }

    Exp = mybir.ActivationFunctionType.Exp
    SCALE = float(D) ** -0.5

    with tile.TileContext(nc, trace_sim=trace_sim) as tc, ExitStack() as ctx:
        kilo = ctx.enter_context(tc.tile_pool(name="kilo", bufs=6))      # xT
        qkp = ctx.enter_context(tc.tile_pool(name="qk", bufs=4))
        wqkp = ctx.enter_context(tc.tile_pool(name="wqk", bufs=2))
        bigp = ctx.enter_context(tc.tile_pool(name="big", bufs=2))       # wv, pT
        vp = ctx.enter_context(tc.tile_pool(name="v", bufs=8))
        ptp = ctx.enter_context(tc.tile_pool(name="pt", bufs=18))
        obp = ctx.enter_context(tc.tile_pool(name="ob", bufs=1))         # o_big
        otp = ctx.enter_context(tc.tile_pool(name="ot", bufs=1))         # oT
        rsp = ctx.enter_context(tc.tile_pool(name="rs", bufs=1))
        accp = ctx.enter_context(tc.tile_pool(name="acc", bufs=8))
        onep = ctx.enter_context(tc.tile_pool(name="one", bufs=1))
        ps_s = ctx.enter_context(tc.tile_pool(name="pss", bufs=2, space="PSUM"))
        ps_acc = ctx.enter_context(tc.tile_pool(name="psa", bufs=2, space="PSUM"))
        ps_row = ctx.enter_context(tc.tile_pool(name="psr", bufs=1, space="PSUM"))
        ps_misc = ctx.enter_context(tc.tile_pool(name="psm", bufs=1, space="PSUM"))

        # warm the ACT exp table set while input DMAs run (the first real exp
        # otherwise pays the ~2.7us ACT_TABLE_LOAD on the critical path)
        warm = onep.tile([1, 4], f32)
        nc.vector.memset(warm[:], 0.0)
        warm2 = onep.tile([1, 4], f32)
        nc.scalar.activation(warm2[:], warm[:], Exp)

        # ---- load constants / inputs ----
        xs = []
        for k in range(KT):
            t = kilo.tile([P, N], bf16, tag="kilo")
            eng = nc.sync if k % 2 == 0 else nc.gpsimd
            eng.dma_start(t[:, 0:512], xT[k][:, 0:512])
            xs.append(t)

        wq_tiles = {}

        def prefetch_wq(t):
            if t not in wq_tiles:
                wq_t = wqkp.tile([P, KT * 256], bf16, tag="wqk", name=f"wq_{t}")
                nc.sync.dma_start(wq_t[:], wqk[t])
                wq_tiles[t] = wq_t

        prefetch_wq(0)
        for k in range(KT):
            eng = nc.sync if k % 2 == 0 else nc.gpsimd
            eng.dma_start(xs[k][:, 512:1024], xT[k][:, 512:1024])

        wv_sb = bigp.tile([P, KT, C], bf16, tag="big")
        nc.gpsimd.dma_start(wv_sb[:], wv[:])

        ones_sb = onep.tile([P, 1], bf16)
        nc.vector.memset(ones_sb[:], 1.0)
        bias_sb = onep.tile([P, C], f32)
        nc.gpsimd.dma_start(bias_sb[:], bias[:])

        qt_sb, kt_sb = [], []

        def emit_qk_one(t, which, store):
            """Q^T or K^T of pair t -> [128 o, 1024 n] bf16, via a ps_s slot."""
            wq_t = wq_tiles[t]
            slot = ps_s.tile([P, 1024], f32, tag="pss", name=f"qk_{t}_{which}")
            for ns in range(2):
                dst = slot[:, ns * 512 : (ns + 1) * 512]
                for k in range(KT):
                    nc.tensor.matmul(
                        dst,
                        wq_t[:, k * 256 + which * P : k * 256 + (which + 1) * P],
                        xs[k][:, ns * 512 : (ns + 1) * 512],
                        start=(k == 0),
                        stop=(k == KT - 1),
                    )
            qk_t = qkp.tile([P, N], bf16, tag="qk")
            nc.vector.tensor_copy(qk_t[:], slot[:, 0:1024])
            store.append(qk_t)

        def emit_qk_misc(t, which, store, half):
            """Half of a Q^T/K^T tile via the single-bank misc slot."""
            wq_t = wq_tiles[t]
            slot = ps_misc.tile([P, 512], f32, tag="psm", name=f"qkm_{t}_{which}_{half}")
            for k in range(KT):
                nc.tensor.matmul(
                    slot[:],
                    wq_t[:, k * 256 + which * P : k * 256 + (which + 1) * P],
                    xs[k][:, half * 512 : (half + 1) * 512],
                    start=(k == 0),
                    stop=(k == KT - 1),
                )
            if half == 0:
                qk_t = qkp.tile([P, N], bf16, tag="qk")
                store.append(qk_t)
            qk_t = store[-1]
            nc.vector.tensor_copy(qk_t[:, half * 512 : (half + 1) * 512], slot[:])

        # ---- V in [m, o] layout (plain), emitted upfront through ps_s ----
        v_sb = []

        def emit_v(nt):
            slot = ps_s.tile([P, 1024], f32, tag="pss", name=f"v_{nt}")
            for c0, w in ((0, 512), (512, 256)):
                for k in range(KT):
                    nc.tensor.matmul(
                        slot[:, c0 : c0 + w],
                        xs[k][:, nt * P : (nt + 1) * P],
                        wv_sb[:, k, c0 : c0 + w],
                        start=(k == 0),
                        stop=(k == KT - 1),
                    )
            vt = vp.tile([P, C], bf16, tag="v")
            nc.vector.tensor_copy(vt[:], slot[:, 0:C])
            v_sb.append(vt)

        # pair-0 QK first, then all of V (PE-bound prologue; ACT is idle
        # anyway, and this keeps the attention inner loop free of V stalls)
        emit_qk_one(0, 0, qt_sb)
        emit_qk_one(0, 1, kt_sb)
        prefetch_wq(1)
        for j in range(NT):
            emit_v(j)

        pt_w = bigp.tile([P, KT, C], bf16, tag="big")
        nc.gpsimd.dma_start(pt_w[:], pT[:])

        # persistent small tiles
        o_big = obp.tile([P, PAIRS, NT, P], bf16, tag="ob")     # [n, cb, nt, c]
        oT = otp.tile([P, PAIRS, NT, P], bf16, tag="ot")        # [c, cb, nt, n]
        rs_sb = rsp.tile([P, H * NT], f32, tag="rs")            # 1/rowsum
        row_ps = ps_row.tile([P, H * NT], f32, tag="psr")       # rowsums

        acc_sb = {}

        def proj_partial(nt):
            """k=0..4 of the projection for n-tile nt (+bias) -> SBUF."""
            acc = accp.tile([P, C], f32, tag="acc", name=f"acc_{nt}")
            for c0, w in ((0, 512), (512, 256)):
                slot = ps_misc.tile([P, w], f32, tag="psm", name=f"pp_{nt}_{c0}")
                for cb in range(KT - 1):
                    nc.tensor.matmul(
                        slot[:],
                        oT[:, cb, nt, :],
                        pt_w[:, cb, c0 : c0 + w],
                        start=(cb == 0),
                        stop=(cb == KT - 2),
                        skip_group_check=True,
                    )
                nc.vector.tensor_add(
                    acc[:, c0 : c0 + w], slot[:], bias_sb[:, c0 : c0 + w]
                )
            acc_sb[nt] = acc

        def proj_finish(nt):
            """k=5 + add + store for n-tile nt (ps_s banks are free by now)."""
            slot = ps_s.tile([P, 1024], f32, tag="pss", name=f"pf_{nt}")
            for c0, w in ((0, 512), (512, 256)):
                nc.tensor.matmul(
                    slot[:, c0 : c0 + w],
                    oT[:, KT - 1, nt, :],
                    pt_w[:, KT - 1, c0 : c0 + w],
                    start=True,
                    stop=True,
                    skip_group_check=True,
                )
            acc = acc_sb.pop(nt)
            nc.vector.tensor_add(acc[:], acc[:], slot[:, 0:C])
            nc.sync.dma_start(y[nt * P : (nt + 1) * P, :], acc[:])

        # ---- attention ----
        # PSUM banks allow only ONE open accumulation group at a time (a
        # start=True resets the bank's accumulation context), so the m-loop
        # of each (head, nt) output group must run back-to-back. Structure:
        # software pipeline with head-slots — slot s computes S+exp of head s
        # while running attn@V of head s-1 against its 8 retained P^T tiles.
        pt_tiles = {}

        def phase1(h, j):
            t, hb = h // 2, (h % 2) * D
            s_slot = ps_s.tile([P, 1024], f32, tag="pss", name=f"s_{h}_{j}")
            for ns in range(2):
                nc.tensor.matmul(
                    s_slot[:, ns * 512 : (ns + 1) * 512],
                    kt_sb[t][hb : hb + D, j * P : (j + 1) * P],
                    qt_sb[t][hb : hb + D, ns * 512 : (ns + 1) * 512],
                    start=True,
                    stop=True,
                )
            pt_t = ptp.tile([P, 1024], bf16, tag="pt", name=f"pt_{h}_{j}")
            nc.scalar.activation(pt_t[:], s_slot[:], Exp, scale=SCALE)
            pt_tiles.setdefault(h, {})[j] = pt_t

        def phase2(h, nt, acc):
            pts = pt_tiles[h]
            for j in range(NT):
                nc.tensor.matmul(
                    acc[:, nt * D : (nt + 1) * D],
                    pts[j][:, nt * P : (nt + 1) * P],
                    v_sb[j][:, h * D : (h + 1) * D],
                    start=(j == 0),
                    stop=(j == NT - 1),
                    skip_group_check=True,
                )
            c = h * NT + nt
            for j in range(NT):
                nc.tensor.matmul(
                    row_ps[:, c : c + 1],
                    pts[j][:, nt * P : (nt + 1) * P],
                    ones_sb[:, 0:1],
                    start=(j == 0),
                    stop=(j == NT - 1),
                    skip_group_check=True,
                )

        def normalize(h, acc):
            t, hb = h // 2, (h % 2) * D
            nc.vector.reciprocal(
                rs_sb[:, h * NT : (h + 1) * NT], row_ps[:, h * NT : (h + 1) * NT]
            )
            rs_base = rs_sb[:, h * NT : (h + 1) * NT]
            if h == H - 1:
                # last head: normalize in nt-halves so the final
                # transpose/proj chain starts earlier
                for g in range(2):
                    half = rs_sb[:, h * NT + g * 4 : h * NT + (g + 1) * 4]
                    rs_half = bass.AP(
                        tensor=half.tensor,
                        offset=half.offset,
                        ap=[list(half.ap[0]), list(half.ap[1]), [0, D]],
                    )
                    nc.vector.tensor_mul(
                        o_big[:, t, g * 4 : (g + 1) * 4, hb : hb + D],
                        acc[:, g * 256 : (g + 1) * 256],
                        rs_half,
                    )
            else:
                rs_bcast = bass.AP(
                    tensor=rs_base.tensor,
                    offset=rs_base.offset,
                    ap=[list(rs_base.ap[0]), list(rs_base.ap[1]), [0, D]],
                )
                nc.vector.tensor_mul(
                    o_big[:, t, :, hb : hb + D], acc[:, 0:512], rs_bcast
                )

        accs = {}
        for slot in range(H + 1):
            h1 = slot        # phase-1 head (S + exp)
            h2 = slot - 1    # phase-2 head (attn@V + rowsums)
            if h2 >= 0:
                accs[h2] = ps_acc.tile([P, 512], f32, tag="psa", name=f"o_{h2}")
            for step in range(NT):
                if h1 < H:
                    phase1(h1, step)
                if h2 >= 0:
                    phase2(h2, step, accs[h2])
                # upcoming QK emission through the misc bank: pair t+1
                # emitted across the two head-slots of pair t
                t = slot // 2
                if slot < 2 * PAIRS and t + 1 < PAIRS:
                    if slot % 2 == 0 and step == 1:
                        emit_qk_misc(t + 1, 0, qt_sb, 0)
                    elif slot % 2 == 0 and step == 4:
                        emit_qk_misc(t + 1, 0, qt_sb, 1)
                    elif slot % 2 == 1 and step == 0:
                        if t + 2 < PAIRS:
                            prefetch_wq(t + 2)
                        emit_qk_misc(t + 1, 1, kt_sb, 0)
                    elif slot % 2 == 1 and step == 3:
                        emit_qk_misc(t + 1, 1, kt_sb, 1)
                # projection partial (k=0..4) overlaps the last two slots
                if slot >= H - 1 and step in (1, 3, 5, 7):
                    proj_partial((slot - (H - 1)) * 4 + step // 2)
            if h2 >= 0:
                normalize(h2, accs[h2])
                del pt_tiles[h2]
                if h2 % 2 == 1:
                    t = h2 // 2
                    if t == PAIRS - 1:
                        for g in range(2):
                            nc.sync.dma_start_transpose(
                                oT[:, t, g * 4 : (g + 1) * 4, :],
                                o_big[:, t, g * 4 : (g + 1) * 4, :],
                            )
                    else:
                        nc.sync.dma_start_transpose(oT[:, t, :, :], o_big[:, t])

        for nt in range(NT):
            proj_finish(nt)

        if debug:
            nc.sync.dma_start(dbg["dq"][:], qt_sb[0][:])
            nc.sync.dma_start(dbg["dk"][:], kt_sb[0][:])
            nc.sync.dma_start(dbg["dv"][:], v_sb[0][:])
            nc.sync.dma_start(dbg["drs"][:], rs_sb[:])
            nc.sync.dma_start(
                dbg["dob"][:], o_big.rearrange("p a b c -> p (a b c)")
            )
            nc.sync.dma_start(
                dbg["dot"][:], oT.rearrange("p a b c -> p (a b c)")
            )

    _split_multi_waits(nc)
    return nc


def _prep_shared(qkv_w, proj_w, proj_b):
    import ml_dtypes

    bf = ml_dtypes.bfloat16
    f = np.float32
    wqT = qkv_w[0:C].astype(f).T.copy()           # [c, o]
    wkT = qkv_w[C : 2 * C].astype(f).T.copy()
    wvT = qkv_w[2 * C : 3 * C].astype(f).T.copy()

    wqk = np.zeros((PAIRS, P, KT, 256), f)
    for t in range(PAIRS):
        for k in range(KT):
            wqk[t, :, k, 0:P] = wqT[k * P : (k + 1) * P, t * P : (t + 1) * P]
            wqk[t, :, k, P:256] = wkT[k * P : (k + 1) * P, t * P : (t + 1) * P]
    wqk = wqk.reshape(PAIRS, P, KT * 256).astype(bf)

    wvh = wvT.reshape(KT, P, C).transpose(1, 0, 2).astype(bf).copy()
    pTh = proj_w.T.astype(f).reshape(KT, P, C).transpose(1, 0, 2).astype(bf).copy()
    bias_h = np.ascontiguousarray(np.broadcast_to(proj_b.astype(f), (P, C)))
    return wqk, wvh, pTh, bias_h


def kernel(x, qkv_w, proj_w, proj_b):
    import ml_dtypes
    from concourse.bass_utils import run_bass_kernel_spmd

    bf = ml_dtypes.bfloat16
    x = np.asarray(x, np.float32)
    wqk, wvh, pTh, bias_h = _prep_shared(
        np.asarray(qkv_w), np.asarray(proj_w), np.asarray(proj_b)
    )

    if "nc" not in _STATE:
        _STATE["nc"] = _build_nc()
    nc = _STATE["nc"]

    in_maps = []
    for b in range(B):
        xTb = np.ascontiguousarray(x[b].T).reshape(KT, P, N).astype(bf)
        in_maps.append(
            {"xT": xTb, "wqk": wqk, "wv": wvh, "pT": pTh, "bias": bias_h}
        )

    res = run_bass_kernel_spmd(nc, in_maps, core_ids=list(range(B)))
    return np.stack([res.results[b]["y"] for b in range(B)], axis=0)
